# revision 9
# baseline (speedup 1.0000x reference)
"""VQ codebook assignment (ApplyKmeans) on 8 Trainium2 NeuronCores.

tokens[n] = argmin_k ||x_n - c_k||^2
          = argmax_k (x_n.c_k - Cnorm_k/2)        (||x_n||^2 constant per row)

Data-parallel: x sharded along N across 8 cores, C/Cnorm replicated.

Per core (16384 rows, 128 row-tiles of 128 rows):
  - host pre-tiles x^T so each [128d, 128n] stationary tile is contiguous
    (fp16: halves HBM traffic; PSUM accumulates fp32; ~52/131072 argmin
    flips vs the fp32 reference, rel err ~0.0144)
  - warmup: 8 dep-free matmuls over an uninitialized SBUF tile, one per
    PSUM bank, issued first. They execute during the initial DMA wait,
    ramping the PE out of its low p-state, and their start=True writes
    set every PSUM has_written bit - so every real tile can use the
    ACT-copy bias path (ScalarE rewrites the bank to -Cnorm/2, then 8
    start=False matmuls accumulate on top). No bias matmuls needed.
  - per tile: 8 accumulating matmuls (x^T chunk stationary, C chunk
    moving) -> PSUM [128, 300] holds val = x.C - Cnorm/2
  - argmax split across engines so DVE stays under the PE's 1017ns/tile
    budget: Pool folds PSUM 300 -> 150 -> 75 -> 38 by elementwise max
    (fp32-exact, so the max VALUE survives bitwise), DVE does MAX8 on
    the 38-wide fold + FIND_INDEX8 on the original PSUM row (first
    occurrence = reference tie-break). Pool also does the token
    compaction copy; tokens stream out on the scalar ring with small
    final flushes to shorten the tail.
  - group 0's x chunks are split across the sync and vector queues so
    the first tile's data lands ASAP after the framework preamble.

Row interleaving: row-tile t holds rows {p*128 + t}, so the token buffer
[p, t] DMAs out contiguously in original row order.

Walrus only lowers one sync wait per instruction; _hoist_excess_waits
moves Tile's extra waits onto same-engine no-ops at the same program
point. Mid-kernel x loads share the sync HWDGE ring (same-ring
transfers complete in order, so prefetch can't starve urgent loads);
constants and token stores ride the scalar ring. KM_HW_LANES=4 sem
lanes keep the end-of-kernel semaphore drain chain short.
"""

import os
import sys

import numpy as np

if "/opt/trn_rl_repo" not in sys.path:
    sys.path.insert(0, "/opt/trn_rl_repo")

import concourse.bass as bass
import concourse.mybir as mybir
import concourse.tile_sem_assignment as _tsa
from concourse.bass_utils import run_bass_kernel_spmd
from concourse.tile import TileContext

_tsa.NUM_HWDGE_SEMS = int(os.environ.get("KM_HW_LANES", "4"))

# Give each HWDGE ring (SP-issued vs ACT-issued DMAs) a disjoint pool of
# completion lanes. Tile's global round-robin otherwise interleaves the
# two rings onto shared lanes, and the lane-order WAW waits then falsely
# serialize one ring behind the other.
_orig_assign_tick = _tsa.TileClockTick._assign_tick


def _assign_tick_lanepools(self, inst):
    try:
        if isinstance(inst, _tsa.DMAInst) and inst.engine != mybir.EngineType.Pool:
            if not hasattr(self, "_lane_ctr"):
                self._lane_ctr = {}
            eng = inst.engine
            n = _tsa.NUM_HWDGE_SEMS
            half = max(1, n // 2)
            pool = (
                list(range(0, half))
                if eng == mybir.EngineType.Activation
                else list(range(half, n))
            )
            c = self._lane_ctr.get(eng, 0)
            self.next_hw_dma_idx = pool[c % len(pool)]
            self._lane_ctr[eng] = c + 1
    except Exception:
        pass
    return _orig_assign_tick(self, inst)


_tsa.TileClockTick._assign_tick = _assign_tick_lanepools

P = 128
D = 1024
K = 300
NCORES = 8
ROWS = 16384            # rows per core
TILES = ROWS // P       # 128 row-tiles per core
GROUPS = 32             # DMA groups per core (1 group = 1 MB fp16)
TPG = TILES // GROUPS   # 8 row-tiles per group
DCH = D // P            # 8 contraction chunks

F16 = mybir.dt.float16
F32 = mybir.dt.float32
I32 = mybir.dt.int32
U32 = mybir.dt.uint32

# Set by kernel() so test.py can read profiling info.
LAST_RESULT = None


def _ensure_ntff_hook():
    """Install antenv.axon_hooks shim so trace=True works under axon."""
    try:
        from antenv.axon_hooks import get_axon_ntff_profile_hook  # noqa: F401

        return
    except ImportError:
        pass
    import types

    import antenv

    try:
        from trn_agent_boot.trn_boot import _ntff_profile_via_ctypes
    except ImportError:
        return
    mod = types.ModuleType("antenv.axon_hooks")
    _hook = [None]
    mod.set_axon_ntff_profile_hook = lambda h: _hook.__setitem__(0, h)
    mod.get_axon_ntff_profile_hook = lambda: _hook[0]
    sys.modules["antenv.axon_hooks"] = mod
    antenv.axon_hooks = mod
    so = "/opt/axon/libaxon_pjrt.so"
    if os.path.exists(so):
        mod.set_axon_ntff_profile_hook(_ntff_profile_via_ctypes(so))


# Token flush boundaries (exclusive tile index): 16-tile blocks through
# t=112, then 8/6/2 so the final CAST+DMA is tiny and the tail is short.
FLUSH = [16, 32, 48, 64, 80, 96, 112, 120, 126, 128]


def build_nc() -> bass.Bass:
    n_warm = int(os.environ.get("KM_WARM", "8"))
    fold_depth = int(os.environ.get("KM_FOLD", "0"))
    spread0 = bool(int(os.environ.get("KM_SPREAD0", "1")))
    pool_cast = bool(int(os.environ.get("KM_POOL_CAST", "1")))

    nc = bass.Bass()

    xg = nc.declare_dram_parameter("xg", [GROUPS, P, DCH * TPG * P], F16, isOutput=False)
    cons = nc.declare_dram_parameter("cons", [P, DCH * K], F16, isOutput=False)
    biasf = nc.declare_dram_parameter("biasf", [P, K], F32, isOutput=False)
    out = nc.declare_dram_parameter("out", [P, TILES], I32, isOutput=True)

    # fold widths per depth: 300 -> 150 -> 75 -> 38 (odd splits overlap
    # one element; max is idempotent so coverage stays exact)
    FW = [150, 75, 38]
    foff = [0]
    for w in FW:
        foff.append(foff[-1] + w)

    with TileContext(nc) as tc:
        with (
            tc.tile_pool(name="const", bufs=1) as constp,
            tc.tile_pool(name="warm", bufs=1) as warmp,
            tc.tile_pool(name="xp0", bufs=DCH) as xp0,
            tc.tile_pool(name="xp", bufs=3) as xp,
            tc.tile_pool(name="fold", bufs=4) as foldp,
            tc.tile_pool(name="mx", bufs=8) as mxp,
            tc.tile_pool(name="psum", bufs=8, space="PSUM") as psp,
            tc.tile_pool(name="outp", bufs=1) as outp,
        ):
            # scalar ring: bias first (gates tile 0's ACT copy), then C
            # in three pieces so ct0 lands before the later chunks
            bft = constp.tile([P, K], F32)
            nc.scalar.dma_start(out=bft[:], in_=biasf[:])
            cons_t = constp.tile([P, DCH * K], F16)
            for lo, hi in ((0, 1), (1, 3), (3, 6), (6, 8)):
                nc.scalar.dma_start(
                    out=cons_t[:, lo * K : hi * K], in_=cons[:, lo * K : hi * K]
                )
            ctiles = [cons_t[:, j * K : (j + 1) * K] for j in range(DCH)]

            # PE warmup: dep-free matmuls over an uninitialized SBUF
            # tile, one per PSUM bank. They run during the startup DMA
            # wait (ramping the PE p-state) and their start=True writes
            # set every bank's has_written bits.
            warm = warmp.tile([P, max(K, P)], F16)
            nc.gpsimd.memset(warm[:], 0.0)
            for w in range(n_warm):
                wps = psp.tile([P, K], F32, name="ps")
                nc.tensor.matmul(
                    wps[:], lhsT=warm[:, :P], rhs=warm[:, :K],
                    start=True, stop=True,
                )

            # group 0 arrives chunk-by-chunk so the PE can start early.
            # Pairing chunks halves the sync-queue issue instructions
            # (~600ns each), so group 1's prefetch issues ~2.4us sooner.
            xch0 = []
            cw = 2 if spread0 else 1
            for j2 in range(DCH // cw):
                cbuf = xp0.tile([P, cw, TPG, P], F16, name="xchunk")
                nc.sync.dma_start(
                    out=cbuf[:],
                    in_=xg[0, :, j2 * cw * TPG * P : (j2 + 1) * cw * TPG * P].rearrange(
                        "p (j t q) -> p j t q", j=cw, t=TPG
                    ),
                )
                xch0.append(cbuf)

            idxbuf = outp.tile([P, TILES, 8], U32)
            tokbuf = outp.tile([P, TILES], I32)

            for g in range(GROUPS):
                if g == 0:
                    chunk = lambda j, tl: xch0[j // cw][:, j % cw, tl, :]
                else:
                    # all steady-state x loads share the sync ring:
                    # same-ring transfers serialize per DMA-engine FIFO,
                    # so prefetch can't steal bandwidth from earlier
                    # (more urgent) loads
                    xbuf = xp.tile([P, DCH, TPG, P], F16, name="xgrp")
                    nc.sync.dma_start(
                        out=xbuf[:],
                        in_=xg[g].rearrange("p (j t q) -> p j t q", j=DCH, t=TPG),
                    )
                    chunk = lambda j, tl, xbuf=xbuf: xbuf[:, j, tl, :]
                for tl in range(TPG):
                    t = g * TPG + tl
                    psum = psp.tile([P, K], F32, name="ps")
                    # has_written bits persist from this bank's previous
                    # occupant (warmup or prior tile); ScalarE resets the
                    # values to the bias and the start=False matmuls
                    # accumulate on top
                    nc.scalar.copy(out=psum[:], in_=bft[:])
                    for j in range(DCH):
                        nc.tensor.matmul(
                            psum[:],
                            lhsT=chunk(j, tl),
                            rhs=ctiles[j][:],
                            start=False,
                            stop=(j == DCH - 1),
                            skip_group_check=True,
                        )
                    # Pool folds the row by elementwise max (values stay
                    # exact fp32), DVE finds top-8 of the fold + the
                    # index in the original row
                    if fold_depth > 0:
                        fold = foldp.tile([P, foff[-1]], F32)
                        src = psum[:, : 2 * FW[0]]
                        half = FW[0]
                        nc.gpsimd.tensor_max(
                            fold[:, : FW[0]], src[:, :half], src[:, half:]
                        )
                        for lv in range(1, fold_depth):
                            prev = fold[:, foff[lv - 1] : foff[lv]]
                            w = FW[lv]
                            pw = FW[lv - 1]
                            nc.gpsimd.tensor_max(
                                fold[:, foff[lv] : foff[lv] + w],
                                prev[:, :w],
                                prev[:, pw - w : pw],
                            )
                        mxsrc = fold[:, foff[fold_depth - 1] : foff[fold_depth]]
                    else:
                        mxsrc = psum[:]
                    mx = mxp.tile([P, 8], F32)
                    nc.vector.max(out=mx[:], in_=mxsrc)
                    nc.vector.max_index(
                        out=idxbuf[:, t, :], in_max=mx[:], in_values=psum[:]
                    )
                    # stream tokens out (scalar ring: the out-DMA's wait
                    # must not block pending x loads)
                    if (t + 1) in FLUSH:
                        s = FLUSH[FLUSH.index(t + 1) - 1] if (t + 1) != FLUSH[0] else 0
                        cast_eng = nc.gpsimd if pool_cast else nc.vector
                        cast_eng.tensor_copy(
                            out=tokbuf[:, s : t + 1], in_=idxbuf[:, s : t + 1, 0]
                        )
                        nc.scalar.dma_start(
                            out=out[:, s : t + 1], in_=tokbuf[:, s : t + 1]
                        )

    _hoist_excess_waits(nc)
    return nc


def _hoist_excess_waits(nc: bass.Bass, max_waits: int = 1):
    """Hoist excess sync waits onto no-op drains inserted just before.

    Walrus's codegen caps embedded sync waits per instruction (1 for
    DIRECT2D DMAs and CTRL ops), but Tile can attach several (slot-reuse
    WAR + lane WAW, or the kernel-tail drain waiting on every proc).
    A same-engine drain immediately before the instruction blocks the
    sequencer at the same program point, so semantics are unchanged.
    """
    n = 0
    for f in nc.m.functions:
        for blk in f.blocks:
            insts = blk.instructions
            i = 0
            while i < len(insts):
                inst = insts[i]
                si = inst.sync_info
                if si and si.on_wait and len(si.on_wait) > max_waits:
                    waits = list(si.on_wait)
                    si.on_wait = waits[-max_waits:]
                    inst.sync_info = si
                    pre = []
                    for j in range(0, len(waits) - max_waits, max_waits):
                        nd = mybir.InstNoOp(name=f"I-wsplit{n}", ins=[], outs=[])
                        n += 1
                        nd.engine = inst.engine
                        nsi = type(si)(
                            on_wait=waits[j : j + max_waits], on_update=[]
                        )
                        nd.sync_info = nsi
                        try:
                            nc.register_instruction(nd, overwrite=True)
                        except Exception:
                            pass
                        pre.append(nd)
                    for k, nd in enumerate(pre):
                        insts.insert(i + k, nd)
                    i += len(pre)
                i += 1


def make_in_maps(x: np.ndarray, C: np.ndarray, Cnorm: np.ndarray):
    x16 = x.astype(np.float16)
    C16 = C.astype(np.float16).reshape(DCH, P, K)

    cons = np.ascontiguousarray(
        C16.transpose(1, 0, 2).reshape(P, DCH * K)
    )
    biasf = np.broadcast_to(
        (-0.5 * Cnorm.reshape(K)).astype(np.float32), (P, K)
    ).copy()

    in_maps = []
    for c in range(NCORES):
        xs = x16[c * ROWS : (c + 1) * ROWS]
        # row r = p*128 + g*TPG + tl ; col = j*128 + pd
        xr = xs.reshape(P, GROUPS, TPG, DCH, P)          # [p, g, tl, j, pd]
        xgc = np.ascontiguousarray(xr.transpose(1, 4, 3, 2, 0))  # [g, pd, j, tl, p]
        in_maps.append(
            {
                "xg": xgc.reshape(GROUPS, P, DCH * TPG * P),
                "cons": cons,
                "biasf": biasf,
            }
        )
    return in_maps


_NC_CACHE = {}


def kernel(x, C, Cnorm, b, t):
    global LAST_RESULT
    x = np.asarray(x)
    C = np.asarray(C)
    Cnorm = np.asarray(Cnorm)

    key = 0
    if key not in _NC_CACHE:
        _NC_CACHE[key] = build_nc()
    nc = _NC_CACHE[key]

    in_maps = make_in_maps(x, C, Cnorm)
    trace = bool(int(os.environ.get("KM_TRACE", "0")))
    if trace:
        _ensure_ntff_hook()
    res = run_bass_kernel_spmd(
        nc, in_maps, core_ids=list(range(NCORES)), trace=trace
    )
    LAST_RESULT = res

    shards = [res.results[c]["out"].reshape(-1) for c in range(NCORES)]
    tokens = np.concatenate(shards).astype(np.int32)
    return tokens.reshape(int(b), int(t))
